# revision 3
# baseline (speedup 1.0000x reference)
"""Trainium2 Bass kernel for the 2D-GRU clause-pair network.

Network (per batch b):
  doc  = hidden_state[b, bert_clause_b[b]]            [N, FEAT]
  r    = doc @ W_red + b_red                          [N, H]
  X    = relu(r_i @ W1 + r_j @ W2 + b')               [N, N, H]   (W1/W2 = W_cat halves)
  Tm   = 2D-GRU wavefront scan over (i, j)            [N, N, H]
  couples = Tm @ W_t + b_t ; pred_e/c = diag(Tm) @ W_e/c + b_e/c

Sharding: data-parallel over batch, 4 batches per core on 8 NeuronCores.
On-core layout is feature-major ("transposed"): every tensor is
[128 partitions = H-features (2 chunks), cells] where cells = (i, b_local).
The scan walks 127 anti-diagonals; each diagonal's cells are independent and
batched into single matmuls (moving dim = 4*n_d <= 256).  The per-cell GRU
state lives in an i-indexed SBUF buffer with a zero guard column so the
left/up fused state is one shifted vector add.

All scan math runs in DT_SCAN (fp16 by default: PE runs 16-bit at 1 cyc/row
vs 4 for fp32, DVE gets 2x mode); PSUM accumulation is always fp32.
"""

import numpy as np

import concourse.mybir as mybir
from concourse import bacc
from concourse.tile import TileContext
from concourse.bass_utils import run_bass_kernel_spmd

B, N, T_TOK, FEAT, H = 32, 64, 512, 768, 256
NCORES = 8
BL = B // NCORES            # 4 batches per core
P = 128
NDIAG = 2 * N - 1           # 127
G3 = 3 * H                  # 768 gate rows

f32 = mybir.dt.float32
f16 = mybir.dt.float16
bf16 = mybir.dt.bfloat16
ALU = mybir.AluOpType
ACTF = mybir.ActivationFunctionType

DT_SCAN = f16               # scan dtype: f16 | bf16 | f32
NP_SCAN = {f16: np.float16, bf16: None, f32: np.float32}

# diagonal geometry -----------------------------------------------------------
DIAG = []
_off = 0
for d in range(NDIAG):
    i0 = max(0, d - (N - 1))
    i1 = min(d, N - 1) + 1
    n = i1 - i0
    DIAG.append((i0, i1, n, _off))
    _off += n * BL
TOT_CELLS = _off            # 4 * 4096 = 16384


def _np_dt(dt):
    return np.dtype(mybir.dt.np(dt))


def build_program(dt_scan=DT_SCAN):
    nc = bacc.Bacc("TRN2", target_bir_lowering=False, debug=False)
    DT = dt_scan
    NPDT = _np_dt(DT)

    # ---- DRAM I/O (per core) ----
    doc_t = nc.dram_tensor("doc_t", [FEAT, N * BL], f32, kind="ExternalInput")
    w_red = nc.dram_tensor("w_red", [FEAT, H], f32, kind="ExternalInput")
    w1 = nc.dram_tensor("w1", [H, H], f32, kind="ExternalInput")
    w2 = nc.dram_tensor("w2", [H, H], f32, kind="ExternalInput")
    wx = nc.dram_tensor("wx", [H, G3], DT, kind="ExternalInput")
    uzr = nc.dram_tensor("uzr", [H, 2 * H], DT, kind="ExternalInput")   # pre-scaled by 0.5
    uh = nc.dram_tensor("uh", [H, H], DT, kind="ExternalInput")         # pre-scaled by 0.5
    wt = nc.dram_tensor("wt", [H, 2], DT, kind="ExternalInput")
    we = nc.dram_tensor("we", [H, 2], DT, kind="ExternalInput")
    wc = nc.dram_tensor("wc", [H, 2], DT, kind="ExternalInput")
    cx = nc.dram_tensor("cx", [H], f32, kind="ExternalInput")           # b_red@(W1+W2)+b_cat
    bg = nc.dram_tensor("bg", [G3], f32, kind="ExternalInput")
    bt = nc.dram_tensor("bt", [2], f32, kind="ExternalInput")
    be = nc.dram_tensor("be", [2], f32, kind="ExternalInput")
    bc = nc.dram_tensor("bc", [2], f32, kind="ExternalInput")

    cpl_stage = nc.dram_tensor("cpl_stage", [2, TOT_CELLS], f32, kind="ExternalOutput")
    pe_stage = nc.dram_tensor("pe_stage", [2, N * BL], f32, kind="ExternalOutput")
    pc_stage = nc.dram_tensor("pc_stage", [2, N * BL], f32, kind="ExternalOutput")

    with TileContext(nc) as tc:
        with (
            tc.tile_pool(name="wpool", bufs=1) as wp,
            tc.tile_pool(name="xpool", bufs=3) as xp,
            tc.tile_pool(name="ppool", bufs=2, space="PSUM") as pp,
            tc.tile_pool(name="cpool", bufs=2, space="PSUM") as cp,
        ):
            # ---- persistent SBUF residents ----
            wx_t = wp.tile([P, 2, 6, P], DT)        # (p, kc, gc, m)
            uzr_t = wp.tile([P, 2, 4, P], DT)
            uh_t = wp.tile([P, 2, 2, P], DT)
            wt_t = wp.tile([P, 2, 2], DT)
            we_t = wp.tile([P, 2, 2], DT)
            wc_t = wp.tile([P, 2, 2], DT)
            cx_t = wp.tile([P, 2], f32)
            bg_t = wp.tile([P, 6], f32)
            bt_t = wp.tile([2, 1], f32)
            be_t = wp.tile([2, 1], f32)
            bc_t = wp.tile([2, 1], f32)

            nc.sync.dma_start(out=wx_t[:], in_=wx.rearrange("(kc p) (gc m) -> p kc gc m", p=P, m=P))
            nc.sync.dma_start(out=uzr_t[:], in_=uzr.rearrange("(kc p) (gc m) -> p kc gc m", p=P, m=P))
            nc.sync.dma_start(out=uh_t[:], in_=uh.rearrange("(kc p) (gc m) -> p kc gc m", p=P, m=P))
            nc.sync.dma_start(out=wt_t[:], in_=wt.rearrange("(kc p) c -> p kc c", p=P))
            nc.sync.dma_start(out=we_t[:], in_=we.rearrange("(kc p) c -> p kc c", p=P))
            nc.sync.dma_start(out=wc_t[:], in_=wc.rearrange("(kc p) c -> p kc c", p=P))
            nc.sync.dma_start(out=cx_t[:], in_=cx.rearrange("(c p) -> p c", p=P))
            nc.sync.dma_start(out=bg_t[:], in_=bg.rearrange("(c p) -> p c", p=P))
            nc.sync.dma_start(out=bt_t[:], in_=bt.rearrange("(c o) -> c o", o=1))
            nc.sync.dma_start(out=be_t[:], in_=be.rearrange("(c o) -> c o", o=1))
            nc.sync.dma_start(out=bc_t[:], in_=bc.rearrange("(c o) -> c o", o=1))

            ai_t = wp.tile([P, 2, N, BL], DT)       # r @ W1, feature-major
            ajr_t = wp.tile([P, 2, N, BL], DT)      # r @ W2, i-reversed
            s_t = wp.tile([P, 2, N + 1, BL], DT)    # state, col 0 = zero guard
            diag_t = wp.tile([P, 2, N, BL], DT)     # states of cells (i, i)

            # ---- phase A: doc -> r -> A_i / A_j ----
            wred_t = wp.tile([P, 6, 2, P], f32)
            w1_t = wp.tile([P, 2, 2, P], f32)
            w2_t = wp.tile([P, 2, 2, P], f32)
            doc_tt = wp.tile([P, 6, N * BL], f32)
            r_t = wp.tile([P, 2, N * BL], f32)
            aj_tmp = xp.tile([P, 2, N, BL], DT, tag="ajtmp")

            nc.sync.dma_start(out=wred_t[:], in_=w_red.rearrange("(kc p) (mc m) -> p kc mc m", p=P, m=P))
            nc.sync.dma_start(out=w1_t[:], in_=w1.rearrange("(kc p) (mc m) -> p kc mc m", p=P, m=P))
            nc.sync.dma_start(out=w2_t[:], in_=w2.rearrange("(kc p) (mc m) -> p kc mc m", p=P, m=P))
            nc.sync.dma_start(out=doc_tt[:], in_=doc_t.rearrange("(kc p) x -> p kc x", p=P))

            for mc in range(2):
                acc = pp.tile([P, N * BL], f32, tag="zb0")
                for kc in range(6):
                    nc.tensor.matmul(acc[:], wred_t[:, kc, mc, :], doc_tt[:, kc, :],
                                     start=(kc == 0), stop=(kc == 5))
                nc.vector.tensor_copy(out=r_t[:, mc, :], in_=acc[:])
            for mc in range(2):
                acc = pp.tile([P, N * BL], f32, tag="zb1")
                for kc in range(2):
                    nc.tensor.matmul(acc[:], w1_t[:, kc, mc, :], r_t[:, kc, :],
                                     start=(kc == 0), stop=(kc == 1))
                nc.vector.tensor_copy(out=ai_t[:, mc, :, :].rearrange("p i b -> p (i b)"), in_=acc[:])
            for mc in range(2):
                acc = pp.tile([P, N * BL], f32, tag="hb")
                for kc in range(2):
                    nc.tensor.matmul(acc[:], w2_t[:, kc, mc, :], r_t[:, kc, :],
                                     start=(kc == 0), stop=(kc == 1))
                nc.vector.tensor_copy(out=aj_tmp[:, mc, :, :].rearrange("p i b -> p (i b)"), in_=acc[:])
            # reverse A_j along i (one-time, on otherwise-idle gpsimd)
            for i in range(N):
                nc.gpsimd.tensor_copy(out=ajr_t[:, :, N - 1 - i, :], in_=aj_tmp[:, :, i, :])

            nc.gpsimd.memset(s_t[:], 0.0)

            # ---- phase B: wavefront scan over 127 anti-diagonals ----
            for d in range(NDIAG):
                i0, i1, n, off = DIAG[d]
                rj = (N - 1) - d + i0            # base col in reversed-A_j for this diag
                M = n * BL

                # X = relu(A_i + A_jrev + c_X)  (feature-major, per H-chunk for bias)
                x_t = xp.tile([P, 2, N, BL], DT, tag="xt")
                for hc in range(2):
                    nc.vector.scalar_tensor_tensor(
                        out=x_t[:, hc, 0:n, :],
                        in0=ai_t[:, hc, i0:i1, :],
                        scalar=cx_t[:, hc:hc + 1],
                        in1=ajr_t[:, hc, rj:rj + n, :],
                        op0=ALU.add, op1=ALU.add)
                nc.vector.tensor_scalar(
                    out=x_t[:, :, 0:n, :], in0=x_t[:, :, 0:n, :],
                    scalar1=0.0, scalar2=None, op0=ALU.max)

                # fused prev state (unscaled sum; 0.5 folded into uzr/uh/blend)
                sp_t = xp.tile([P, 2, N, BL], DT, tag="sp")
                nc.vector.tensor_add(
                    out=sp_t[:, :, 0:n, :],
                    in0=s_t[:, :, i0:i1, :],
                    in1=s_t[:, :, i0 + 1:i1 + 1, :])

                # gates: psum[gc] = sum_kc Wx[kc,gc].T @ X[kc] (+ recurrent part)
                zb1 = pp.tile([P, 2, N, BL], f32, tag="zb1")   # gates 256:511 (r-gate)
                zb0 = pp.tile([P, 2, N, BL], f32, tag="zb0")   # gates 0:255   (z-gate)
                hb = pp.tile([P, 2, N, BL], f32, tag="hb")     # gates 512:767 (h cand)
                for g in range(2):                              # r-gate first (critical path)
                    for kc in range(2):
                        nc.tensor.matmul(zb1[:, g, 0:n, :], wx_t[:, kc, 2 + g, :],
                                         x_t[:, kc, 0:n, :], start=(kc == 0), stop=False)
                    for kc in range(2):
                        nc.tensor.matmul(zb1[:, g, 0:n, :], uzr_t[:, kc, 2 + g, :],
                                         sp_t[:, kc, 0:n, :], start=False, stop=(kc == 1))
                rr_t = xp.tile([P, 2, N, BL], DT, tag="rr")
                for g in range(2):
                    nc.scalar.activation(out=rr_t[:, g, 0:n, :], in_=zb1[:, g, 0:n, :],
                                         func=ACTF.Sigmoid, bias=bg_t[:, 2 + g:3 + g])
                rh_t = xp.tile([P, 2, N, BL], DT, tag="rh")
                nc.vector.tensor_mul(out=rh_t[:, :, 0:n, :], in0=rr_t[:, :, 0:n, :],
                                     in1=sp_t[:, :, 0:n, :])
                for g in range(2):                              # h candidate
                    for kc in range(2):
                        nc.tensor.matmul(hb[:, g, 0:n, :], wx_t[:, kc, 4 + g, :],
                                         x_t[:, kc, 0:n, :], start=(kc == 0), stop=False)
                    for kc in range(2):
                        nc.tensor.matmul(hb[:, g, 0:n, :], uh_t[:, kc, g, :],
                                         rh_t[:, kc, 0:n, :], start=False, stop=(kc == 1))
                for g in range(2):                              # z gate
                    for kc in range(2):
                        nc.tensor.matmul(zb0[:, g, 0:n, :], wx_t[:, kc, g, :],
                                         x_t[:, kc, 0:n, :], start=(kc == 0), stop=False)
                    for kc in range(2):
                        nc.tensor.matmul(zb0[:, g, 0:n, :], uzr_t[:, kc, g, :],
                                         sp_t[:, kc, 0:n, :], start=False, stop=(kc == 1))
                h_t = xp.tile([P, 2, N, BL], DT, tag="ht")
                z_t = xp.tile([P, 2, N, BL], DT, tag="zt")
                for g in range(2):
                    nc.scalar.activation(out=h_t[:, g, 0:n, :], in_=hb[:, g, 0:n, :],
                                         func=ACTF.Tanh, bias=bg_t[:, 4 + g:5 + g])
                for g in range(2):
                    nc.scalar.activation(out=z_t[:, g, 0:n, :], in_=zb0[:, g, 0:n, :],
                                         func=ACTF.Sigmoid, bias=bg_t[:, g:g + 1])

                # s = 0.5*sp + z*(h - 0.5*sp)   (via v = 0.5*sp - h; s = 0.5*sp - z*v)
                v_t = xp.tile([P, 2, N, BL], DT, tag="vt")
                nc.vector.scalar_tensor_tensor(
                    out=v_t[:, :, 0:n, :], in0=sp_t[:, :, 0:n, :], scalar=0.5,
                    in1=h_t[:, :, 0:n, :], op0=ALU.mult, op1=ALU.subtract)
                nc.vector.tensor_mul(out=v_t[:, :, 0:n, :], in0=z_t[:, :, 0:n, :],
                                     in1=v_t[:, :, 0:n, :])
                nc.vector.scalar_tensor_tensor(
                    out=s_t[:, :, i0 + 1:i1 + 1, :], in0=sp_t[:, :, 0:n, :], scalar=0.5,
                    in1=v_t[:, :, 0:n, :], op0=ALU.mult, op1=ALU.subtract)

                # couples_pred for this diagonal: W_t.T @ s + b_t
                cb = cp.tile([2, N * BL], f32, tag="cb")
                for kc in range(2):
                    nc.tensor.matmul(cb[:, 0:M], wt_t[:, kc, :],
                                     s_t[:, kc, i0 + 1:i1 + 1, :],
                                     start=(kc == 0), stop=(kc == 1))
                cpl_sb = xp.tile([2, N * BL], f32, tag="cpl")
                nc.scalar.activation(out=cpl_sb[:, 0:M], in_=cb[:, 0:M],
                                     func=ACTF.Identity, bias=bt_t[:, 0:1])
                nc.sync.dma_start(out=cpl_stage[:, off:off + M], in_=cpl_sb[:, 0:M])

                if d % 2 == 0:                   # cell (i, i) lives on diagonal 2i
                    i = d // 2
                    nc.gpsimd.tensor_copy(out=diag_t[:, :, i, :],
                                          in_=s_t[:, :, i + 1:i + 2, :])

            # ---- phase C: pred_e / pred_c from diagonal states ----
            for w_sb, b_sb, stage in ((we_t, be_t, pe_stage), (wc_t, bc_t, pc_stage)):
                acc = cp.tile([2, N * BL], f32, tag="cb")
                for kc in range(2):
                    nc.tensor.matmul(acc[:], w_sb[:, kc, :],
                                     diag_t[:, kc, :, :].rearrange("p i b -> p (i b)"),
                                     start=(kc == 0), stop=(kc == 1))
                out_sb = xp.tile([2, N * BL], f32, tag="cpl")
                nc.scalar.activation(out=out_sb[:], in_=acc[:],
                                     func=ACTF.Identity, bias=b_sb[:, 0:1])
                nc.sync.dma_start(out=stage[:], in_=out_sb[:])

    nc.compile()
    return nc


_PROGRAM_CACHE = {}


def _get_program(dt_scan=DT_SCAN):
    key = str(dt_scan)
    if key not in _PROGRAM_CACHE:
        _PROGRAM_CACHE[key] = build_program(dt_scan)
    return _PROGRAM_CACHE[key]


def _host_prep(inputs, dt_scan=DT_SCAN):
    """Shard + lay out inputs for the 8 cores. Returns (in_maps, shared)."""
    NPDT = _np_dt(dt_scan)
    hs = np.asarray(inputs["hidden_state"], np.float32)
    idx = np.asarray(inputs["bert_clause_b"], np.int32)
    W_red = np.asarray(inputs["W_red"], np.float32)
    b_red = np.asarray(inputs["b_red"], np.float32)
    W_cat = np.asarray(inputs["W_cat"], np.float32)
    b_cat = np.asarray(inputs["b_cat"], np.float32)
    W_x = np.asarray(inputs["W_x"], np.float32)
    Us_zr = np.asarray(inputs["Us_zr"], np.float32)
    Us_h = np.asarray(inputs["Us_h"], np.float32)
    b_g = np.asarray(inputs["b_g"], np.float32)
    W_e = np.asarray(inputs["W_e"], np.float32)
    b_e = np.asarray(inputs["b_e"], np.float32)
    W_c = np.asarray(inputs["W_c"], np.float32)
    b_c = np.asarray(inputs["b_c"], np.float32)
    W_t = np.asarray(inputs["W_t"], np.float32)
    b_t = np.asarray(inputs["b_t"], np.float32)

    W1, W2 = W_cat[:H], W_cat[H:]
    shared = {
        "w_red": np.ascontiguousarray(W_red),
        "w1": np.ascontiguousarray(W1),
        "w2": np.ascontiguousarray(W2),
        "wx": np.ascontiguousarray(W_x.astype(NPDT)),
        "uzr": np.ascontiguousarray((0.5 * Us_zr).astype(NPDT)),
        "uh": np.ascontiguousarray((0.5 * Us_h).astype(NPDT)),
        "wt": np.ascontiguousarray(W_t.astype(NPDT)),
        "we": np.ascontiguousarray(W_e.astype(NPDT)),
        "wc": np.ascontiguousarray(W_c.astype(NPDT)),
        "cx": (b_red @ (W1 + W2) + b_cat).astype(np.float32),
        "bg": np.ascontiguousarray(b_g),
        "bt": np.ascontiguousarray(b_t),
        "be": np.ascontiguousarray(b_e),
        "bc": np.ascontiguousarray(b_c),
    }

    doc = np.take_along_axis(hs, idx[..., None], axis=1)        # [B, N, FEAT]
    in_maps = []
    for c in range(NCORES):
        doc_c = doc[c * BL:(c + 1) * BL]                        # [BL, N, FEAT]
        doc_tc = np.ascontiguousarray(
            doc_c.transpose(2, 1, 0).reshape(FEAT, N * BL), np.float32)
        m = dict(shared)
        m["doc_t"] = doc_tc
        in_maps.append(m)
    return in_maps


# host-side unpermutation of the diag-major staging buffers --------------------
_POS = np.empty((BL, N, N), np.int64)
for _d in range(NDIAG):
    _i0, _i1, _n, _o = DIAG[_d]
    for _p, _i in enumerate(range(_i0, _i1)):
        _j = _d - _i
        _POS[:, _i, _j] = _o + _p * BL + np.arange(BL)

last_run_info = {}


def kernel(**inputs):
    nc = _get_program(DT_SCAN)
    in_maps = _host_prep(inputs, DT_SCAN)
    import time
    t0 = time.time()
    res = run_bass_kernel_spmd(nc, in_maps, core_ids=list(range(NCORES)))
    last_run_info["wall_s"] = time.time() - t0
    last_run_info["exec_time_ns"] = res.exec_time_ns
    last_run_info["mean_exec_time_ns"] = res.mean_exec_time_ns

    couples = np.empty((B, N, N, 2), np.float32)
    pred_e = np.empty((B, N, 2), np.float32)
    pred_c = np.empty((B, N, 2), np.float32)
    for c in range(NCORES):
        r = res.results[c]
        sl = slice(c * BL, (c + 1) * BL)
        cs = r["cpl_stage"]                                    # [2, TOT_CELLS]
        couples[sl] = cs[:, _POS].transpose(1, 2, 3, 0)         # [BL,N,N,2]
        pe = r["pe_stage"].reshape(2, N, BL)                    # [2, N, BL]
        pred_e[sl] = pe.transpose(2, 1, 0)
        pc = r["pc_stage"].reshape(2, N, BL)
        pred_c[sl] = pc.transpose(2, 1, 0)
    return couples, pred_e, pred_c
